# revision 3
# baseline (speedup 1.0000x reference)
"""Distance-aware comb-pilot interpolator for Trainium2 (8 NeuronCores).

Math: out[b, 8k+r, c] = alpha[r]*H[b,k,c] + gamma[r]*H[b,k+1,c] with pilots on
the comb loc[k] = 8k (k = 0..511), Nfft = 4096.  Since alpha+gamma = 1 (up to
the reference's 1e-12 eps), rewrite as out = H_k + gamma[r]*D_k with
D_k = H_{k+1} - H_k.  That turns the per-output work into one tensor_scalar
mul (DVE 4x mode / ACT 1x) plus one tensor_tensor add (DVE 2x mode) instead
of the 1x-only fused scalar_tensor_tensor the previous version used.

Everything on device is fp16: the host quantizes LS_ri to fp16 (rel err
~1e-3, far under the 2e-2 gate) and upcasts the fp16 output back to fp32.
This halves HBM traffic (1.05 MB in + 8.39 MB out per core ~= 26.4 us at
358 GB/s) and doubles DVE throughput.  Compute: DVE ~25 us busy (t0 subs,
3/8 of the muls at 4x, all adds at 2x), ACT ~21 us (5/8 of the muls),
GPSIMD ~10 us (2 loads, 3 D-subs, last-block tail) -- all under the DMA
floor, so the kernel is memory-bound as intended.

The last 8 subcarriers (i = 4088..4095) interpolate between pilot 511 at
4088 and a virtual pilot hN = 1.875*H[511] - 0.875*H[510] at 4095 (gap 7,
not 8), giving per-r coefficients on H[510]/H[511] directly; they are three
tiny broadcast ops on GPSIMD per batch tile.
"""

import sys

import numpy as np

for _p in ("/opt/trn_rl_repo", "/root/.axon_site/_ro/trn_rl_repo"):
    if _p not in sys.path:
        sys.path.append(_p)

import concourse.bass as bass
import concourse.tile as tile
from concourse import bacc, mybir
from concourse.bass_utils import run_bass_kernel_spmd

N_CORES = 8
B, NP, NFFT, SPACING = 4096, 512, 4096, 8
B_LOC = B // N_CORES  # batch rows per core
NSEG = NP - 1  # regular 8-wide segments (k = 0..510)
P = 128  # SBUF partitions
N_BT = B_LOC // P  # 128-batch tiles per core

_PROGRAM = None


def _build_program():
    """One Bass program, identical on all cores (pure data parallel)."""
    nc = bacc.Bacc("TRN2", target_bir_lowering=False, debug=False)
    f16 = mybir.dt.float16
    f32 = mybir.dt.float32
    ls = nc.dram_tensor("ls", [B_LOC, NP * 2], f16, kind="ExternalInput").ap()
    cg = nc.dram_tensor("cg", [P, 8], f32, kind="ExternalInput").ap()
    cl = nc.dram_tensor("cl", [P, 32], f16, kind="ExternalInput").ap()
    out = nc.dram_tensor("out", [B_LOC, NFFT * 2], f16, kind="ExternalOutput").ap()

    # Per-tile k-chunking: chunk boundaries gate the add ops and the store
    # DMAs.  Tile 0 is chunked fine so the store stream ramps early; tile 3
    # has a small trailing chunk so the post-compute drain is short.  Muls
    # are issued lazily at the first chunk of their MRANGE; the tile-0 lead
    # range runs entirely on DVE (no cross-engine dep on the critical path),
    # elsewhere ACT takes A_RS and DVE takes V_RS of the 8 r-phases.
    CHUNKS = {
        0: [(0, 128), (128, 320), (320, NSEG)],
        1: [(0, NSEG)],
        2: [(0, NSEG)],
        3: [(0, 384), (384, NSEG)],
    }
    # mul/D ranges (each CHUNK lies inside exactly one MRANGE)
    MRANGES = {
        0: [(0, 128), (128, NSEG)],
        1: [(0, NSEG)],
        2: [(0, NSEG)],
        3: [(0, NSEG)],
    }
    V_RS = (5, 6, 7)  # r-phases whose mul runs on DVE (tensor_scalar 4x)
    A_RS = (0, 1, 2, 3, 4)  # r-phases whose mul runs on ACT

    with tile.TileContext(nc) as tc:
        with (
            tc.tile_pool(name="cpool", bufs=1) as cpool,
            tc.tile_pool(name="hpool", bufs=4) as hpool,
            tc.tile_pool(name="dpool", bufs=6) as dpool,
            tc.tile_pool(name="tpool", bufs=12) as tpool,
            tc.tile_pool(name="opool", bufs=4) as opool,
            tc.tile_pool(name="lpool", bufs=8) as lpool,
        ):
            # Loads: h0 first on ACT's HWDGE ring (fast first byte, gates the
            # whole pipeline), the tiny gamma tile right after; ctl + h1 on
            # SP's HWDGE (idle until the first store); h2/h3 via gpsimd SWDGE.
            hs = [hpool.tile([P, NP * 2], f16, name=f"h{t}", tag="h") for t in range(N_BT)]
            nc.scalar.dma_start(hs[0][:], ls[0:P, :])
            ct = cpool.tile([P, 8], f32)
            nc.scalar.dma_start(ct[:], cg)
            ctl = cpool.tile([P, 32], f16)
            nc.sync.dma_start(ctl[:], cl)
            nc.sync.dma_start(hs[1][:], ls[P : 2 * P, :])
            nc.gpsimd.dma_start(hs[2][:], ls[2 * P : 3 * P, :])
            nc.gpsimd.dma_start(hs[3][:], ls[3 * P : 4 * P, :])

            a_last = ctl[:, 0:16].rearrange("p (r c) -> p r c", c=2)
            c_last = ctl[:, 16:32].rearrange("p (r c) -> p r c", c=2)

            def hseg(t, k0, k1):
                """[P, k1-k0, 2] dense view of pilots k0..k1-1 of tile t."""
                return hs[t][:, 2 * k0 : 2 * k1].rearrange("p (k c) -> p k c", c=2)

            for t in range(N_BT):
                o = opool.tile([P, NFFT * 2], f16, name=f"o{t}", tag="o")
                ov = o[:].rearrange("p (k r c) -> p k r c", r=SPACING, c=2)

                # D = H[k+1]-H[k] per mrange.  Tile 0 on DVE (it gates the
                # whole ACT mul chain); tiles 1-3 on the mostly-idle GPSIMD,
                # well ahead of when ACT/DVE need them.
                ds = {}
                deng = nc.vector if t == 0 else nc.gpsimd
                for m0, m1 in MRANGES[t]:
                    d = dpool.tile([P, 2 * NSEG], f16, name=f"d{t}_{m0}", tag="d")
                    deng.tensor_sub(
                        d[:, 0 : 2 * (m1 - m0)],
                        hs[t][:, 2 * m0 + 2 : 2 * m1 + 2],
                        hs[t][:, 2 * m0 : 2 * m1],
                    )
                    ds[m0] = d

                # Last 8 subcarriers on GPSIMD (tiny broadcast ops), written
                # into o's tail; covered by the tile's final chunk store.
                h510 = hs[t][:, 2 * NP - 4 : 2 * NP - 2].unsqueeze(1).broadcast_to((P, 8, 2))
                h511 = hs[t][:, 2 * NP - 2 : 2 * NP].unsqueeze(1).broadcast_to((P, 8, 2))
                tl = lpool.tile([P, 8, 2], f16, name=f"tl{t}", tag="tl")
                nc.gpsimd.tensor_mul(tl[:], h510, a_last)
                t2 = lpool.tile([P, 8, 2], f16, name=f"t2{t}", tag="t2")
                nc.gpsimd.tensor_mul(t2[:], h511, c_last)
                o_last = o[:, NSEG * 16 : NFFT * 2].rearrange("p (r c) -> p r c", c=2)
                nc.gpsimd.tensor_add(o_last, tl[:], t2[:])

                # chunks: issue the mrange's muls lazily, then the 8 adds,
                # then the store.  Add order: DVE-produced tmps first, then
                # ACT's in production order (minimizes cross-engine stall).
                tmps = {}
                for ci, (k0, k1) in enumerate(CHUNKS[t]):
                    last = ci == len(CHUNKS[t]) - 1
                    m0, m1 = next(m for m in MRANGES[t] if m[0] <= k0 and k1 <= m[1])
                    all_dve = t == 0 and m0 == 0
                    order = list(V_RS) + list(A_RS)
                    if (m0, order[0]) not in tmps:
                        w = 2 * (m1 - m0)
                        for r in order:
                            tmp = tpool.tile(
                                [P, 2 * NSEG], f16, name=f"tmp{t}_{m0}_{r}", tag="tmp"
                            )
                            if all_dve or r in V_RS:
                                nc.vector.tensor_scalar_mul(
                                    tmp[:, 0:w], ds[m0][:, 0:w], ct[:, r : r + 1]
                                )
                            else:
                                nc.scalar.mul(tmp[:, 0:w], ds[m0][:, 0:w], ct[:, r : r + 1])
                            tmps[(m0, r)] = tmp

                    for r in order:
                        tv = tmps[(m0, r)][:, 2 * (k0 - m0) : 2 * (k1 - m0)].rearrange(
                            "p (k c) -> p k c", c=2
                        )
                        nc.vector.tensor_add(ov[:, k0:k1, r, :], tv, hseg(t, k0, k1))

                    lo = k0 * 16
                    hi = NFFT * 2 if last else k1 * 16
                    nc.sync.dma_start(out[t * P : (t + 1) * P, lo:hi], o[:, lo:hi])
    nc.compile()
    return nc


def _coefs(decay_param: np.ndarray):
    """gamma [128,8] f32; last-block coefs on H510/H511 [128,32] f16."""
    x = np.float32(np.asarray(decay_param).reshape(-1)[0])
    d = np.logaddexp(np.float32(0.0), x, dtype=np.float32)  # softplus
    r = np.arange(SPACING, dtype=np.float32)
    eps = np.float32(1e-12)
    wl = np.exp(-d * r, dtype=np.float32)
    wr = np.exp(-d * (np.float32(SPACING) - r), dtype=np.float32)
    gamma = wr / (wl + wr + eps)
    # last chunk: i = 4088 + r, x0 = 4088, x1 = 4095 (gap of 7);
    # y1 = hN = 1.875*H[511] - 0.875*H[510]
    wl2 = np.exp(-d * r, dtype=np.float32)
    wr2 = np.exp(-d * (np.float32(7.0) - r), dtype=np.float32)
    w2 = wl2 + wr2 + eps
    c511 = (wl2 + np.float32(1.875) * wr2) / w2
    c510 = -np.float32(0.875) * wr2 / w2
    cg = np.broadcast_to(gamma, (P, 8)).astype(np.float32).copy()
    row = np.concatenate([np.repeat(c510, 2), np.repeat(c511, 2)])
    cl = np.broadcast_to(row, (P, 32)).astype(np.float16).copy()
    return cg, cl


def kernel(LS_ri, pilot_pos=None, decay_param=None, Nfft=None, **_unused):
    global _PROGRAM
    ls16 = np.ascontiguousarray(
        np.asarray(LS_ri, dtype=np.float32).reshape(B, NP * 2).astype(np.float16)
    )
    cg, cl = _coefs(decay_param)

    if _PROGRAM is None:
        _PROGRAM = _build_program()
    nc = _PROGRAM

    in_maps = []
    for c in range(N_CORES):
        in_maps.append(
            {"ls": ls16[c * B_LOC : (c + 1) * B_LOC], "cg": cg, "cl": cl}
        )

    res = run_bass_kernel_spmd(nc, in_maps, list(range(N_CORES))).results
    out = np.concatenate(
        [res[c]["out"].astype(np.float32).reshape(B_LOC, NFFT, 2) for c in range(N_CORES)],
        axis=0,
    )
    return out


# revision 4
# speedup vs baseline: 1.3517x; 1.3517x over previous
"""Distance-aware comb-pilot interpolator for Trainium2 (8 NeuronCores).

Math: out[b, 8k+r, c] = alpha[r]*H[b,k,c] + gamma[r]*H[b,k+1,c], pilots on the
comb loc[k] = 8k (k = 0..511), Nfft = 4096.  Two identities cut the work:

  alpha[r] + gamma[r] = 1  (up to the reference's 1e-12 eps)
      -> out_r = H_k + gamma[r]*D_k  with  D = H[k+1] - H[k]
  alpha[8-r] = gamma[r]  (weight symmetry of the exp-decay kernel)
      -> out_{8-r} = H_{k+1} - gamma[r]*D_k   (reuses the same product)

So per batch-tile only 5 tensor_scalar muls (r = 0..4, on ACT mostly) and 8
dense tensor add/sub ops (DVE 2x fp16 mode) produce all 4096 subcarriers.

The device computes fp16 end-to-end and writes the output in r-major order
out_dev[b, r, k, c] so every DVE op touches only unit-stride APs (strided
dst was measured to drop DVE to 1x mode); the host de-interleaves with a
numpy transpose and upcasts to fp32 (rel err ~8e-4, gate is 2e-2).

HBM traffic per core: 1.05 MB in + 8.39 MB out ~= 26.4 us at 358 GB/s.
Engine busy: DVE ~24 us, ACT ~20 us, GPSIMD ~13 us (loads h2/h3, D-subs for
tiles 1-3, the 16-col last-block tail) -- memory-bound as intended.

The last 8 subcarriers (i = 4088..4095) interpolate between pilot 511 at
4088 and a virtual pilot hN = 1.875*H[511] - 0.875*H[510] at 4095 (gap 7,
not 8): per-r coefficients on H[510]/H[511], three tiny broadcast GPSIMD
ops per tile writing o[:, r, 511, :].
"""

import sys

import numpy as np

for _p in ("/opt/trn_rl_repo", "/root/.axon_site/_ro/trn_rl_repo"):
    if _p not in sys.path:
        sys.path.append(_p)

import concourse.bass as bass
import concourse.tile as tile
from concourse import bacc, mybir
from concourse.bass_utils import run_bass_kernel_spmd

N_CORES = 8
B, NP, NFFT, SPACING = 4096, 512, 4096, 8
B_LOC = B // N_CORES  # batch rows per core
NSEG = NP - 1  # regular 8-wide segments (k = 0..510)
P = 128  # SBUF partitions
N_BT = B_LOC // P  # 128-batch tiles per core
RW = 2 * NP  # o-tile columns per r-phase (512 k-slots x 2)

_PROGRAM = None


def _build_program():
    """One Bass program, identical on all cores (pure data parallel)."""
    nc = bacc.Bacc("TRN2", target_bir_lowering=False, debug=False)
    f16 = mybir.dt.float16
    f32 = mybir.dt.float32
    ls = nc.dram_tensor("ls", [B_LOC, NP * 2], f16, kind="ExternalInput").ap()
    cg = nc.dram_tensor("cg", [P, 8], f32, kind="ExternalInput").ap()
    cl = nc.dram_tensor("cl", [P, 32], f16, kind="ExternalInput").ap()
    out = nc.dram_tensor("out", [B_LOC, NFFT * 2], f16, kind="ExternalOutput").ap()

    # Mul engine per (tile, r): 'V' entries run on DVE (tensor_scalar, 4x
    # mode, no cross-engine latency -- used at the ramp (t0) and the drain
    # (t3 r4)); everything else on ACT.  Adds are issued right after the mul
    # they consume; the r-major store groups are ordered by completion.
    DVE_MULS = {(0, 0), (0, 1), (3, 4)}
    ADD_ORDER = [0, 1, 7, 2, 6, 3, 5, 4]  # add_r after mul_min(r,8-r)
    SGROUPS = {
        0: [(0, 1), (1, 2), (6, 8), (2, 4), (4, 6)],
        1: [(0, 2), (6, 8), (2, 4), (4, 6)],
        2: [(0, 2), (6, 8), (2, 4), (4, 6)],
        3: [(0, 2), (6, 8), (2, 4), (5, 6), (4, 5)],
    }

    with tile.TileContext(nc) as tc:
        with (
            tc.tile_pool(name="cpool", bufs=1) as cpool,
            tc.tile_pool(name="hpool", bufs=4) as hpool,
            tc.tile_pool(name="dpool", bufs=4) as dpool,
            tc.tile_pool(name="tpool", bufs=10) as tpool,
            tc.tile_pool(name="opool", bufs=4) as opool,
            tc.tile_pool(name="lpool", bufs=8) as lpool,
        ):
            # Loads: h0 first on ACT's HWDGE ring (fast first byte, gates the
            # whole pipeline), the tiny gamma tile right after; ctl + h1 on
            # SP's HWDGE (idle until the first store); h2/h3 via gpsimd SWDGE.
            hs = [hpool.tile([P, NP * 2], f16, name=f"h{t}", tag="h") for t in range(N_BT)]
            nc.scalar.dma_start(hs[0][:], ls[0:P, :])
            ct = cpool.tile([P, 8], f32)
            nc.scalar.dma_start(ct[:], cg)
            ctl = cpool.tile([P, 32], f16)
            nc.sync.dma_start(ctl[:], cl)
            nc.sync.dma_start(hs[1][:], ls[P : 2 * P, :])
            nc.gpsimd.dma_start(hs[2][:], ls[2 * P : 3 * P, :])
            nc.gpsimd.dma_start(hs[3][:], ls[3 * P : 4 * P, :])

            a_last = ctl[:, 0:16].rearrange("p (r c) -> p r c", c=2)
            c_last = ctl[:, 16:32].rearrange("p (r c) -> p r c", c=2)

            for t in range(N_BT):
                h = hs[t]
                o = opool.tile([P, NFFT * 2], f16, name=f"o{t}", tag="o")
                orv = o[:].rearrange("p (r k) -> p r k", r=SPACING)

                # D = H[k+1]-H[k]: tile 0 on DVE (it gates the whole mul
                # chain), tiles 1-3 on the mostly-idle GPSIMD, well ahead.
                d = dpool.tile([P, 2 * NSEG], f16, name=f"d{t}", tag="d")
                (nc.vector if t == 0 else nc.gpsimd).tensor_sub(
                    d[:], h[:, 2 : 2 * NP], h[:, 0 : 2 * NSEG]
                )

                # Last 8 subcarriers -> o[:, r, 511, :] (tiny GPSIMD ops).
                h510 = h[:, 2 * NP - 4 : 2 * NP - 2].unsqueeze(1).broadcast_to((P, 8, 2))
                h511 = h[:, 2 * NP - 2 : 2 * NP].unsqueeze(1).broadcast_to((P, 8, 2))
                tl = lpool.tile([P, 8, 2], f16, name=f"tl{t}", tag="tl")
                nc.gpsimd.tensor_mul(tl[:], h510, a_last)
                t2 = lpool.tile([P, 8, 2], f16, name=f"t2{t}", tag="t2")
                nc.gpsimd.tensor_mul(t2[:], h511, c_last)
                o_last = orv[:, :, 2 * NSEG : 2 * NP]
                nc.gpsimd.tensor_add(o_last, tl[:], t2[:])

                # 5 muls (tmp_r = gamma_r * D) + 8 dense adds/subs.
                tmps = {}
                for r in ADD_ORDER:
                    m = min(r, SPACING - r)
                    if m not in tmps:
                        tmp = tpool.tile([P, 2 * NSEG], f16, name=f"tmp{t}_{m}", tag="tmp")
                        if (t, m) in DVE_MULS:
                            nc.vector.tensor_scalar_mul(tmp[:], d[:], ct[:, m : m + 1])
                        else:
                            nc.scalar.mul(tmp[:], d[:], ct[:, m : m + 1])
                        tmps[m] = tmp
                    dst = orv[:, r, 0 : 2 * NSEG]
                    if r <= 4:
                        nc.vector.tensor_add(dst, tmps[m][:], h[:, 0 : 2 * NSEG])
                    else:
                        nc.vector.tensor_sub(dst, h[:, 2 : 2 * NP], tmps[m][:])

                for r0, r1 in SGROUPS[t]:
                    nc.sync.dma_start(
                        out[t * P : (t + 1) * P, r0 * RW : r1 * RW],
                        o[:, r0 * RW : r1 * RW],
                    )
    nc.compile()
    return nc


def _coefs(decay_param: np.ndarray):
    """gamma [128,8] f32; last-block coefs on H510/H511 [128,32] f16."""
    x = np.float32(np.asarray(decay_param).reshape(-1)[0])
    d = np.logaddexp(np.float32(0.0), x, dtype=np.float32)  # softplus
    r = np.arange(SPACING, dtype=np.float32)
    eps = np.float32(1e-12)
    wl = np.exp(-d * r, dtype=np.float32)
    wr = np.exp(-d * (np.float32(SPACING) - r), dtype=np.float32)
    gamma = wr / (wl + wr + eps)
    # last block: i = 4088 + r, x0 = 4088, x1 = 4095 (gap of 7);
    # y1 = hN = 1.875*H[511] - 0.875*H[510]
    wl2 = np.exp(-d * r, dtype=np.float32)
    wr2 = np.exp(-d * (np.float32(7.0) - r), dtype=np.float32)
    w2 = wl2 + wr2 + eps
    c511 = (wl2 + np.float32(1.875) * wr2) / w2
    c510 = -np.float32(0.875) * wr2 / w2
    cg = np.broadcast_to(gamma, (P, 8)).astype(np.float32).copy()
    row = np.concatenate([np.repeat(c510, 2), np.repeat(c511, 2)])
    cl = np.broadcast_to(row, (P, 32)).astype(np.float16).copy()
    return cg, cl


def kernel(LS_ri, pilot_pos=None, decay_param=None, Nfft=None, **_unused):
    global _PROGRAM
    ls16 = np.ascontiguousarray(
        np.asarray(LS_ri, dtype=np.float32).reshape(B, NP * 2).astype(np.float16)
    )
    cg, cl = _coefs(decay_param)

    if _PROGRAM is None:
        _PROGRAM = _build_program()
    nc = _PROGRAM

    in_maps = []
    for c in range(N_CORES):
        in_maps.append(
            {"ls": ls16[c * B_LOC : (c + 1) * B_LOC], "cg": cg, "cl": cl}
        )

    res = run_bass_kernel_spmd(nc, in_maps, list(range(N_CORES))).results
    # device output is r-major [b, r, k, c]; de-interleave to [b, 8k+r, c]
    out = np.concatenate(
        [
            res[c]["out"]
            .reshape(B_LOC, SPACING, NP, 2)
            .transpose(0, 2, 1, 3)
            .reshape(B_LOC, NFFT, 2)
            for c in range(N_CORES)
        ],
        axis=0,
    ).astype(np.float32)
    return out
